# revision 1
# baseline (speedup 1.0000x reference)
"""BitNetLinear on 8 Trainium2 NeuronCores.

Computes out = x @ sign(weight).T + bias for x[4,2048,4096] f32,
weight[4096,4096] f32, bias[4096] f32.

Strategy: 4-way data parallel over rows x 2-way tensor parallel over
out_features (each core owns a [2048, 2048] block of the [8192, 4096]
output; no collectives, host stitches blocks).

Per core the matmul runs in fp16 hi/lo split: x = hi + lo with both
halves fp16 (sign(weight) is exactly representable in fp16), both
passes accumulated into the same PSUM banks in fp32. This gives
~fp32 accuracy (measured max err / scale ~5e-7 vs float64) at bf16
matmul speed: 1 PE cycle per moving row vs 4 for true fp32.

Layouts are precomputed on the host so every DMA is contiguous:
  xt[mt, d, db*128+m] = x_pass[m0 + mt*128 + m, db*128 + d]
  wt[db, d, o]        = sign(weight)[o0 + o, db*128 + d]
The kernel keeps all 32 weight blocks resident in SBUF (128 KB per
partition), streams x tiles (one 1 MB contiguous DMA per pass per
m-tile), and accumulates each [128, 512] output chunk over 64 matmuls
(2 passes x 32 k-blocks) before a DVE eviction fused with the bias add.
"""

import numpy as np

import concourse.mybir as mybir
import concourse.tile as tile
from concourse import bacc
from concourse.bass_utils import run_bass_kernel_spmd

B, S, D_IN, D_OUT = 4, 2048, 4096, 4096
M_TOT = B * S  # 8192
N_CORES = 8
MG, OG = 4, 2  # data-parallel row groups x tensor-parallel out_feature groups
M_SH = M_TOT // MG  # 2048 rows per core
O_SH = D_OUT // OG  # 2048 out features per core
P = 128
DB = D_IN // P  # 32 contraction blocks
MT = M_SH // P  # 16 m-tiles per core
NF = 512  # moving free dim per matmul (one PSUM bank of fp32)
NCH = O_SH // NF  # 4 output chunks per m-tile

_CACHE = {}


def _build():
    nc = bacc.Bacc("TRN2", target_bir_lowering=False, debug=False)
    xt_hi_d = nc.dram_tensor(
        "xt_hi", [MT, P, DB * P], mybir.dt.float16, kind="ExternalInput"
    )
    xt_lo_d = nc.dram_tensor(
        "xt_lo", [MT, P, DB * P], mybir.dt.float16, kind="ExternalInput"
    )
    wt_d = nc.dram_tensor("wt", [DB, P, O_SH], mybir.dt.float16, kind="ExternalInput")
    bias_d = nc.dram_tensor("biasb", [P, O_SH], mybir.dt.float32, kind="ExternalInput")
    out_d = nc.dram_tensor("out", [M_SH, O_SH], mybir.dt.float32, kind="ExternalOutput")

    with tile.TileContext(nc) as tc:
        with (
            tc.tile_pool(name="wpool", bufs=1) as wpool,
            tc.tile_pool(name="xpool", bufs=2) as xpool,
            tc.tile_pool(name="opool", bufs=3) as opool,
            tc.tile_pool(name="psum", bufs=2, space="PSUM") as psum_pool,
        ):
            bias_sb = wpool.tile([P, O_SH], mybir.dt.float32, name="bias_sb")
            nc.sync.dma_start(out=bias_sb[:], in_=bias_d[:])
            w_sb = []
            for db in range(DB):
                w = wpool.tile(
                    [P, O_SH], mybir.dt.float16, name=f"w{db}", tag=f"w{db}"
                )
                nc.sync.dma_start(out=w[:], in_=wt_d[db])
                w_sb.append(w)
            for mt in range(MT):
                x_hi = xpool.tile([P, DB * P], mybir.dt.float16, name="x_hi", tag="xhi")
                x_lo = xpool.tile([P, DB * P], mybir.dt.float16, name="x_lo", tag="xlo")
                nc.sync.dma_start(out=x_hi[:], in_=xt_hi_d[mt])
                nc.sync.dma_start(out=x_lo[:], in_=xt_lo_d[mt])
                psums = [
                    psum_pool.tile(
                        [P, NF], mybir.dt.float32, name=f"ps{oc}", tag=f"ps{oc}"
                    )
                    for oc in range(NCH)
                ]
                for i, x_sb in enumerate((x_hi, x_lo)):
                    for db in range(DB):
                        lhsT = x_sb[:, db * P : (db + 1) * P]
                        first = i == 0 and db == 0
                        last = i == 1 and db == DB - 1
                        for oc in range(NCH):
                            nc.tensor.matmul(
                                psums[oc][:],
                                lhsT,
                                w_sb[db][:, oc * NF : (oc + 1) * NF],
                                start=first,
                                stop=last,
                            )
                for oc in range(NCH):
                    o_sb = opool.tile(
                        [P, NF], mybir.dt.float32, name="o_sb", tag=f"o{oc}"
                    )
                    nc.vector.tensor_add(
                        o_sb[:], psums[oc][:], bias_sb[:, oc * NF : (oc + 1) * NF]
                    )
                    nc.sync.dma_start(
                        out=out_d[mt * P : (mt + 1) * P, oc * NF : (oc + 1) * NF],
                        in_=o_sb[:],
                    )
    nc.compile()
    return nc


def _prep_inputs(x, weight, bias):
    xf = np.ascontiguousarray(x.reshape(M_TOT, D_IN), dtype=np.float32)
    x_hi = xf.astype(np.float16)
    x_lo = (xf - x_hi.astype(np.float32)).astype(np.float16)

    qw = np.sign(weight.astype(np.float32)).astype(np.float16)  # [o, d]

    # per o-group weight block + broadcast bias, shared by all cores in group
    wt_og, bias_og = [], []
    for og in range(OG):
        o0 = og * O_SH
        blk = np.ascontiguousarray(qw[o0 : o0 + O_SH, :].T)  # [d, o]
        wt_og.append(blk.reshape(DB, P, O_SH))
        bb = np.ascontiguousarray(
            np.broadcast_to(bias[o0 : o0 + O_SH].astype(np.float32), (P, O_SH))
        )
        bias_og.append(bb)

    in_maps = []
    for c in range(N_CORES):
        mg, og = c // OG, c % OG
        m0 = mg * M_SH
        xt = {}
        for name, arr in (("xt_hi", x_hi), ("xt_lo", x_lo)):
            r = arr[m0 : m0 + M_SH].reshape(MT, P, DB, P)  # [mt, m, db, d]
            xt[name] = np.ascontiguousarray(r.transpose(0, 3, 2, 1)).reshape(
                MT, P, DB * P
            )
        in_maps.append(
            {
                "xt_hi": xt["xt_hi"],
                "xt_lo": xt["xt_lo"],
                "wt": wt_og[og],
                "biasb": bias_og[og],
            }
        )
    return in_maps


def run(inputs, trace=False):
    """Run the SPMD kernel; returns (full_output, BassKernelResults)."""
    if "nc" not in _CACHE:
        _CACHE["nc"] = _build()
    nc = _CACHE["nc"]
    in_maps = _prep_inputs(inputs["x"], inputs["weight"], inputs["bias"])
    res = run_bass_kernel_spmd(nc, in_maps, list(range(N_CORES)), trace=trace)
    out = np.empty((M_TOT, D_OUT), dtype=np.float32)
    for c in range(N_CORES):
        mg, og = c // OG, c % OG
        out[mg * M_SH : (mg + 1) * M_SH, og * O_SH : (og + 1) * O_SH] = res.results[
            c
        ]["out"]
    return out.reshape(B, S, D_OUT), res


def kernel(x, weight, bias):
    out, _ = run({"x": x, "weight": weight, "bias": bias})
    return out


# revision 2
# speedup vs baseline: 1.0295x; 1.0295x over previous
"""BitNetLinear on 8 Trainium2 NeuronCores.

Computes out = x @ sign(weight).T + bias for x[4,2048,4096] f32,
weight[4096,4096] f32, bias[4096] f32.

Strategy: 4-way data parallel over rows x 2-way tensor parallel over
out_features (each core owns a [2048, 2048] block of the [8192, 4096]
output; no collectives, host stitches blocks).

Per core the matmul runs in fp16 hi/lo split: x = hi + lo with both
halves fp16 (sign(weight) is exactly representable in fp16), both
passes accumulated into the same PSUM banks in fp32. This gives
~fp32 accuracy (measured max err / scale ~5e-7 vs float64) at bf16
matmul speed: 1 PE cycle per moving row vs 4 for true fp32.

Layouts are precomputed on the host so every DMA is contiguous:
  xt[mt, d, db*128+m] = x_pass[m0 + mt*128 + m, db*128 + d]
  wt[db, d, o]        = sign(weight)[o0 + o, db*128 + d]
The kernel keeps all 32 weight blocks resident in SBUF (128 KB per
partition), streams x tiles (one 1 MB contiguous DMA per pass per
m-tile), and accumulates each [128, 512] output chunk over 64 matmuls
(2 passes x 32 k-blocks) before a DVE eviction fused with the bias add.
"""

import numpy as np

import concourse.mybir as mybir
import concourse.tile as tile
from concourse import bacc
from concourse.bass_utils import run_bass_kernel_spmd

B, S, D_IN, D_OUT = 4, 2048, 4096, 4096
M_TOT = B * S  # 8192
N_CORES = 8
MG, OG = 4, 2  # data-parallel row groups x tensor-parallel out_feature groups
M_SH = M_TOT // MG  # 2048 rows per core
O_SH = D_OUT // OG  # 2048 out features per core
P = 128
DB = D_IN // P  # 32 contraction blocks
MT = M_SH // P  # 16 m-tiles per core
NF = 512  # moving free dim per matmul (one PSUM bank of fp32)
NCH = O_SH // NF  # 4 output chunks per m-tile

_CACHE = {}


def _build():
    nc = bacc.Bacc("TRN2", target_bir_lowering=False, debug=False)
    xt_hi_d = nc.dram_tensor(
        "xt_hi", [MT, P, DB * P], mybir.dt.float16, kind="ExternalInput"
    )
    xt_lo_d = nc.dram_tensor(
        "xt_lo", [MT, P, DB * P], mybir.dt.float16, kind="ExternalInput"
    )
    wt_d = nc.dram_tensor("wt", [DB, P, O_SH], mybir.dt.float16, kind="ExternalInput")
    bias_d = nc.dram_tensor("biasb", [P, O_SH], mybir.dt.float32, kind="ExternalInput")
    out_d = nc.dram_tensor("out", [M_SH, O_SH], mybir.dt.float32, kind="ExternalOutput")

    with tile.TileContext(nc) as tc:
        with (
            tc.tile_pool(name="wpool", bufs=1) as wpool,
            tc.tile_pool(name="xpool", bufs=2) as xpool,
            tc.tile_pool(name="opool", bufs=3) as opool,
            tc.tile_pool(name="psum", bufs=2, space="PSUM") as psum_pool,
        ):

            def load_x(mt):
                x_hi = xpool.tile(
                    [P, DB * P], mybir.dt.float16, name="x_hi", tag="xhi"
                )
                x_lo = xpool.tile(
                    [P, DB * P], mybir.dt.float16, name="x_lo", tag="xlo"
                )
                nc.sync.dma_start(out=x_hi[:], in_=xt_hi_d[mt])
                nc.sync.dma_start(out=x_lo[:], in_=xt_lo_d[mt])
                return x_hi, x_lo

            def alloc_psums(mt):
                return [
                    psum_pool.tile(
                        [P, NF], mybir.dt.float32, name=f"ps{oc}", tag=f"ps{oc}"
                    )
                    for oc in range(NCH)
                ]

            def mm_block(x_pair, psums, db, first, last):
                # 8 matmuls: hi+lo pass over one k-block into 4 psum banks
                for i, x_sb in enumerate(x_pair):
                    lhsT = x_sb[:, db * P : (db + 1) * P]
                    for oc in range(NCH):
                        nc.tensor.matmul(
                            psums[oc][:],
                            lhsT,
                            w_sb[db][:, oc * NF : (oc + 1) * NF],
                            start=first and i == 0,
                            stop=last and i == 1,
                        )

            def evict(mt, psums):
                for oc in range(NCH):
                    o_sb = opool.tile(
                        [P, NF], mybir.dt.float32, name="o_sb", tag=f"o{oc}"
                    )
                    nc.vector.tensor_add(
                        o_sb[:], psums[oc][:], bias_sb[:, oc * NF : (oc + 1) * NF]
                    )
                    nc.sync.dma_start(
                        out=out_d[mt * P : (mt + 1) * P, oc * NF : (oc + 1) * NF],
                        in_=o_sb[:],
                    )

            # Prefetch x for the first two m-tiles BEFORE the 16 MB weight
            # preload so the PE can start as soon as w0 lands.
            x01 = [load_x(0), load_x(1)]
            w_sb = []
            for db in range(DB):
                w = wpool.tile(
                    [P, O_SH], mybir.dt.float16, name=f"w{db}", tag=f"w{db}"
                )
                nc.sync.dma_start(out=w[:], in_=wt_d[db])
                w_sb.append(w)
            bias_sb = wpool.tile([P, O_SH], mybir.dt.float32, name="bias_sb")
            nc.sync.dma_start(out=bias_sb[:], in_=bias_d[:])

            # Startup phase: m-tiles 0+1 jointly, k-block-major, so PE
            # consumption (~3.4us per k-block) paces the weight stream
            # (~3.1us per block) instead of stalling for the full preload.
            psums01 = [alloc_psums(0), alloc_psums(1)]
            for db in range(DB):
                for mt in range(2):
                    mm_block(x01[mt], psums01[mt], db, db == 0, db == DB - 1)
            for mt in range(2):
                evict(mt, psums01[mt])

            # Steady state: one m-tile at a time, double-buffered.
            for mt in range(2, MT):
                x_pair = load_x(mt)
                psums = alloc_psums(mt)
                for db in range(DB):
                    mm_block(x_pair, psums, db, db == 0, db == DB - 1)
                evict(mt, psums)
    nc.compile()
    return nc


def _prep_inputs(x, weight, bias):
    xf = np.ascontiguousarray(x.reshape(M_TOT, D_IN), dtype=np.float32)
    x_hi = xf.astype(np.float16)
    x_lo = (xf - x_hi.astype(np.float32)).astype(np.float16)

    qw = np.sign(weight.astype(np.float32)).astype(np.float16)  # [o, d]

    # per o-group weight block + broadcast bias, shared by all cores in group
    wt_og, bias_og = [], []
    for og in range(OG):
        o0 = og * O_SH
        blk = np.ascontiguousarray(qw[o0 : o0 + O_SH, :].T)  # [d, o]
        wt_og.append(blk.reshape(DB, P, O_SH))
        bb = np.ascontiguousarray(
            np.broadcast_to(bias[o0 : o0 + O_SH].astype(np.float32), (P, O_SH))
        )
        bias_og.append(bb)

    in_maps = []
    for c in range(N_CORES):
        mg, og = c // OG, c % OG
        m0 = mg * M_SH
        xt = {}
        for name, arr in (("xt_hi", x_hi), ("xt_lo", x_lo)):
            r = arr[m0 : m0 + M_SH].reshape(MT, P, DB, P)  # [mt, m, db, d]
            xt[name] = np.ascontiguousarray(r.transpose(0, 3, 2, 1)).reshape(
                MT, P, DB * P
            )
        in_maps.append(
            {
                "xt_hi": xt["xt_hi"],
                "xt_lo": xt["xt_lo"],
                "wt": wt_og[og],
                "biasb": bias_og[og],
            }
        )
    return in_maps


def run(inputs, trace=False):
    """Run the SPMD kernel; returns (full_output, BassKernelResults)."""
    if "nc" not in _CACHE:
        _CACHE["nc"] = _build()
    nc = _CACHE["nc"]
    in_maps = _prep_inputs(inputs["x"], inputs["weight"], inputs["bias"])
    res = run_bass_kernel_spmd(nc, in_maps, list(range(N_CORES)), trace=trace)
    out = np.empty((M_TOT, D_OUT), dtype=np.float32)
    for c in range(N_CORES):
        mg, og = c // OG, c % OG
        out[mg * M_SH : (mg + 1) * M_SH, og * O_SH : (og + 1) * O_SH] = res.results[
            c
        ]["out"]
    return out.reshape(B, S, D_OUT), res


def kernel(x, weight, bias):
    out, _ = run({"x": x, "weight": weight, "bias": bias})
    return out


# revision 15
# speedup vs baseline: 1.0399x; 1.0101x over previous
"""BitNetLinear on 8 Trainium2 NeuronCores.

Computes out = x @ sign(weight).T + bias for x[4,2048,4096] f32,
weight[4096,4096] f32, bias[4096] f32.

Strategy: 4-way data parallel over rows x 2-way tensor parallel over
out_features (each core owns a [2048, 2048] block of the [8192, 4096]
output; no collectives, host stitches blocks).

Per core the matmul runs in fp16 hi/lo split: x = hi + lo with both
halves fp16 (sign(weight) is exactly representable in fp16), both
passes accumulated into the same PSUM banks in fp32. This gives
~fp32 accuracy (measured max err / scale ~5e-7 vs float64) at bf16
matmul speed: 1 PE cycle per moving row vs 4 for true fp32.

Layouts are precomputed on the host so every DMA is contiguous:
  xt[mt, d, db*128+m] = x_pass[m0 + mt*128 + m, db*128 + d]
  wt[db, d, o]        = sign(weight)[o0 + o, db*128 + d]
The kernel keeps all 32 weight blocks resident in SBUF (128 KB per
partition), streams x tiles (one 1 MB contiguous DMA per pass per
m-tile), and accumulates each [128, 512] output chunk over 64 matmuls
(2 passes x 32 k-blocks) before a DVE eviction fused with the bias add.
"""

import numpy as np

import concourse.mybir as mybir
import concourse.tile as tile
from concourse import bacc
from concourse.bass_utils import run_bass_kernel_spmd

B, S, D_IN, D_OUT = 4, 2048, 4096, 4096
M_TOT = B * S  # 8192
N_CORES = 8
MG, OG = 4, 2  # data-parallel row groups x tensor-parallel out_feature groups
M_SH = M_TOT // MG  # 2048 rows per core
O_SH = D_OUT // OG  # 2048 out features per core
P = 128
DB = D_IN // P  # 32 contraction blocks
MT = M_SH // P  # 16 m-tiles per core
NF = 512  # moving free dim per matmul (one PSUM bank of fp32)
NCH = O_SH // NF  # 4 output chunks per m-tile

_CACHE = {}


def _build():
    nc = bacc.Bacc("TRN2", target_bir_lowering=False, debug=False)
    xt_hi_d = nc.dram_tensor(
        "xt_hi", [MT, P, DB * P], mybir.dt.float16, kind="ExternalInput"
    )
    xt_lo_d = nc.dram_tensor(
        "xt_lo", [MT, P, DB * P], mybir.dt.float16, kind="ExternalInput"
    )
    wt_d = nc.dram_tensor("wt", [DB, P, O_SH], mybir.dt.float16, kind="ExternalInput")
    # x for m-tiles 0..1 again, but in k-block-major layout for the startup
    # phase: contiguous [128, 128] blocks ordered (db, mt, pass).
    xt_pair_d = nc.dram_tensor(
        "xt_pair", [DB, 2, 2, P, P], mybir.dt.float16, kind="ExternalInput"
    )
    bias_d = nc.dram_tensor("biasb", [P, O_SH], mybir.dt.float32, kind="ExternalInput")
    out_d = nc.dram_tensor("out", [M_SH, O_SH], mybir.dt.float32, kind="ExternalOutput")

    with tile.TileContext(nc) as tc:
        with (
            tc.tile_pool(name="wpool", bufs=1) as wpool,
            tc.tile_pool(name="xpool", bufs=2) as xpool,
            tc.tile_pool(name="psum", bufs=2, space="PSUM") as psum_pool,
        ):

            def load_x(mt):
                x_hi = xpool.tile(
                    [P, DB * P], mybir.dt.float16, name="x_hi", tag="xhi"
                )
                x_lo = xpool.tile(
                    [P, DB * P], mybir.dt.float16, name="x_lo", tag="xlo"
                )
                nc.sync.dma_start(out=x_hi[:], in_=xt_hi_d[mt])
                nc.sync.dma_start(out=x_lo[:], in_=xt_lo_d[mt])
                return x_hi, x_lo

            def alloc_psums(mt):
                return [
                    psum_pool.tile(
                        [P, NF], mybir.dt.float32, name=f"ps{oc}", tag=f"ps{oc}"
                    )
                    for oc in range(NCH)
                ]

            def mm_block(x_pair, psums, db, first, last):
                # 8 matmuls: hi+lo pass over one k-block into 4 psum banks
                for i, x_sb in enumerate(x_pair):
                    lhsT = x_sb[:, db * P : (db + 1) * P]
                    for oc in range(NCH):
                        nc.tensor.matmul(
                            psums[oc][:],
                            lhsT,
                            w_sb[db][:, oc * NF : (oc + 1) * NF],
                            start=first and i == 0,
                            stop=last and i == 1,
                        )

            def evict(opool, mt, psums):
                for oc in range(NCH):
                    o_sb = opool.tile(
                        [P, NF], mybir.dt.float32, name="o_sb", tag=f"o{oc}"
                    )
                    nc.vector.tensor_add(
                        o_sb[:], psums[oc][:], bias_sb[:, oc * NF : (oc + 1) * NF]
                    )
                    nc.sync.dma_start(
                        out=out_d[mt * P : (mt + 1) * P, oc * NF : (oc + 1) * NF],
                        in_=o_sb[:],
                    )

            w_sb = []
            with tc.tile_pool(name="xstart", bufs=1) as xstart_pool:
                # Startup phase x: per-k-block [128, 128] tiles so the first
                # matmul only waits for ~1.1 MB (w0 + 4 small x blocks), not
                # whole 2 MB x tiles behind the weight stream.
                xs_sb = {}
                for db in range(DB):
                    for mt in range(2):
                        for pi in range(2):
                            t = xstart_pool.tile(
                                [P, P],
                                mybir.dt.float16,
                                name=f"xs{db}_{mt}_{pi}",
                                tag=f"xs{db}_{mt}_{pi}",
                            )
                            nc.sync.dma_start(out=t[:], in_=xt_pair_d[db, mt, pi])
                            xs_sb[db, mt, pi] = t
                    # interleave weight blocks with startup-x so w[db] arrives
                    # roughly when the PE needs it
                    w = wpool.tile(
                        [P, O_SH], mybir.dt.float16, name=f"w{db}", tag=f"w{db}"
                    )
                    nc.sync.dma_start(out=w[:], in_=wt_d[db])
                    w_sb.append(w)
                bias_sb = wpool.tile([P, O_SH], mybir.dt.float32, name="bias_sb")
                nc.sync.dma_start(out=bias_sb[:], in_=bias_d[:])

                # Startup phase: m-tiles 0+1 jointly, k-block-major, so PE
                # consumption (~3.4us per k-block) paces the weight stream
                # (~3.1us per block) instead of stalling for the full preload.
                psums01 = [alloc_psums(0), alloc_psums(1)]
                for db in range(DB):
                    for mt in range(2):
                        for pi in range(2):
                            lhsT = xs_sb[db, mt, pi]
                            for oc in range(NCH):
                                nc.tensor.matmul(
                                    psums01[mt][oc][:],
                                    lhsT[:],
                                    w_sb[db][:, oc * NF : (oc + 1) * NF],
                                    start=db == 0 and pi == 0,
                                    stop=db == DB - 1 and pi == 1,
                                )
            # opool created only after xstart is released so their SBUF
            # address ranges can overlap in time.
            with tc.tile_pool(name="opool", bufs=2) as opool:
                for mt in range(2):
                    evict(opool, mt, psums01[mt])

                # Steady state: one m-tile at a time, double-buffered.
                for mt in range(2, MT):
                    x_pair = load_x(mt)
                    psums = alloc_psums(mt)
                    for db in range(DB):
                        mm_block(x_pair, psums, db, db == 0, db == DB - 1)
                    evict(opool, mt, psums)
    nc.compile()
    return nc


def _prep_inputs(x, weight, bias):
    xf = np.ascontiguousarray(x.reshape(M_TOT, D_IN), dtype=np.float32)
    x_hi = xf.astype(np.float16)
    x_lo = (xf - x_hi.astype(np.float32)).astype(np.float16)

    qw = np.sign(weight.astype(np.float32)).astype(np.float16)  # [o, d]

    # per o-group weight block + broadcast bias, shared by all cores in group
    wt_og, bias_og = [], []
    for og in range(OG):
        o0 = og * O_SH
        blk = np.ascontiguousarray(qw[o0 : o0 + O_SH, :].T)  # [d, o]
        wt_og.append(blk.reshape(DB, P, O_SH))
        bb = np.ascontiguousarray(
            np.broadcast_to(bias[o0 : o0 + O_SH].astype(np.float32), (P, O_SH))
        )
        bias_og.append(bb)

    in_maps = []
    for c in range(N_CORES):
        mg, og = c // OG, c % OG
        m0 = mg * M_SH
        xt = {}
        r4 = {}
        for name, arr in (("xt_hi", x_hi), ("xt_lo", x_lo)):
            r = arr[m0 : m0 + M_SH].reshape(MT, P, DB, P)  # [mt, m, db, d]
            r4[name] = np.ascontiguousarray(r.transpose(0, 3, 2, 1))  # [mt,d,db,m]
            xt[name] = r4[name].reshape(MT, P, DB * P)
        # startup-phase copy of m-tiles 0..1 in k-block-major order
        xt_pair = np.empty((DB, 2, 2, P, P), dtype=np.float16)
        for pi, name in enumerate(("xt_hi", "xt_lo")):
            xt_pair[:, :, pi] = r4[name][:2].transpose(2, 0, 1, 3)  # [db, mt, d, m]
        in_maps.append(
            {
                "xt_hi": xt["xt_hi"],
                "xt_lo": xt["xt_lo"],
                "xt_pair": xt_pair,
                "wt": wt_og[og],
                "biasb": bias_og[og],
            }
        )
    return in_maps


def run(inputs, trace=False):
    """Run the SPMD kernel; returns (full_output, BassKernelResults)."""
    if "nc" not in _CACHE:
        _CACHE["nc"] = _build()
    nc = _CACHE["nc"]
    in_maps = _prep_inputs(inputs["x"], inputs["weight"], inputs["bias"])
    res = run_bass_kernel_spmd(nc, in_maps, list(range(N_CORES)), trace=trace)
    out = np.empty((M_TOT, D_OUT), dtype=np.float32)
    for c in range(N_CORES):
        mg, og = c // OG, c % OG
        out[mg * M_SH : (mg + 1) * M_SH, og * O_SH : (og + 1) * O_SH] = res.results[
            c
        ]["out"]
    return out.reshape(B, S, D_OUT), res


def kernel(x, weight, bias):
    out, _ = run({"x": x, "weight": weight, "bias": bias})
    return out


# revision 17
# speedup vs baseline: 1.0495x; 1.0092x over previous
"""BitNetLinear on 8 Trainium2 NeuronCores.

Computes out = x @ sign(weight).T + bias for x[4,2048,4096] f32,
weight[4096,4096] f32, bias[4096] f32.

Strategy: 4-way data parallel over rows x 2-way tensor parallel over
out_features (each core owns a [2048, 2048] block of the [8192, 4096]
output; no collectives, host stitches blocks).

Per core the matmul runs in fp16 hi/lo split: x = hi + lo with both
halves fp16 (sign(weight) is exactly representable in fp16), both
passes accumulated into the same PSUM banks in fp32. This gives
~fp32 accuracy (measured max err / scale ~5e-7 vs float64) at bf16
matmul speed: 1 PE cycle per moving row vs 4 for true fp32.

Layouts are precomputed on the host so every DMA is contiguous:
  xt[mt, d, db*128+m] = x_pass[m0 + mt*128 + m, db*128 + d]
  wt[db, d, o]        = sign(weight)[o0 + o, db*128 + d]
The kernel keeps all 32 weight blocks resident in SBUF (128 KB per
partition), streams x tiles (one 1 MB contiguous DMA per pass per
m-tile), and accumulates each [128, 512] output chunk over 64 matmuls
(2 passes x 32 k-blocks) before a DVE eviction fused with the bias add.
"""

import numpy as np

import concourse.mybir as mybir
import concourse.tile as tile
from concourse import bacc
from concourse.bass_utils import run_bass_kernel_spmd

B, S, D_IN, D_OUT = 4, 2048, 4096, 4096
M_TOT = B * S  # 8192
N_CORES = 8
MG, OG = 4, 2  # data-parallel row groups x tensor-parallel out_feature groups
M_SH = M_TOT // MG  # 2048 rows per core
O_SH = D_OUT // OG  # 2048 out features per core
P = 128
DB = D_IN // P  # 32 contraction blocks
MT = M_SH // P  # 16 m-tiles per core
NF = 512  # moving free dim per matmul (one PSUM bank of fp32)
NCH = O_SH // NF  # 4 output chunks per m-tile

_CACHE = {}


def _build():
    nc = bacc.Bacc("TRN2", target_bir_lowering=False, debug=False)
    xt_hi_d = nc.dram_tensor(
        "xt_hi", [MT, P, DB * P], mybir.dt.float16, kind="ExternalInput"
    )
    xt_lo_d = nc.dram_tensor(
        "xt_lo", [MT, P, DB * P], mybir.dt.float16, kind="ExternalInput"
    )
    wt_d = nc.dram_tensor("wt", [DB, P, O_SH], mybir.dt.float16, kind="ExternalInput")
    # x for m-tiles 0..1 again, but in k-block-major layout for the startup
    # phase: contiguous [128, 128] blocks ordered (db, mt, pass).
    xt_pair_d = nc.dram_tensor(
        "xt_pair", [DB, 2, 2, P, P], mybir.dt.float16, kind="ExternalInput"
    )
    bias_d = nc.dram_tensor("biasb", [P, O_SH], mybir.dt.float32, kind="ExternalInput")
    out_d = nc.dram_tensor("out", [M_SH, O_SH], mybir.dt.float32, kind="ExternalOutput")

    with tile.TileContext(nc) as tc:
        with (
            tc.tile_pool(name="wpool", bufs=1) as wpool,
            tc.tile_pool(name="xpool", bufs=2) as xpool,
            tc.tile_pool(name="psum", bufs=2, space="PSUM") as psum_pool,
        ):

            def load_x(mt):
                x_hi = xpool.tile(
                    [P, DB * P], mybir.dt.float16, name="x_hi", tag="xhi"
                )
                x_lo = xpool.tile(
                    [P, DB * P], mybir.dt.float16, name="x_lo", tag="xlo"
                )
                nc.sync.dma_start(out=x_hi[:], in_=xt_hi_d[mt])
                nc.sync.dma_start(out=x_lo[:], in_=xt_lo_d[mt])
                return x_hi, x_lo

            def alloc_psums(mt):
                return [
                    psum_pool.tile(
                        [P, NF], mybir.dt.float32, name=f"ps{oc}", tag=f"ps{oc}"
                    )
                    for oc in range(NCH)
                ]

            def mm_block(x_pair, psums, db, first, last):
                # 8 matmuls: hi+lo pass over one k-block into 4 psum banks
                for i, x_sb in enumerate(x_pair):
                    lhsT = x_sb[:, db * P : (db + 1) * P]
                    for oc in range(NCH):
                        nc.tensor.matmul(
                            psums[oc][:],
                            lhsT,
                            w_sb[db][:, oc * NF : (oc + 1) * NF],
                            start=first and i == 0,
                            stop=last and i == 1,
                        )

            def evict(opool, mt, psums):
                for oc in range(NCH):
                    o_sb = opool.tile(
                        [P, NF], mybir.dt.float32, name="o_sb", tag=f"o{oc}"
                    )
                    nc.vector.tensor_add(
                        o_sb[:], psums[oc][:], bias_sb[:, oc * NF : (oc + 1) * NF]
                    )
                    nc.sync.dma_start(
                        out=out_d[mt * P : (mt + 1) * P, oc * NF : (oc + 1) * NF],
                        in_=o_sb[:],
                    )

            w_sb = []
            with tc.tile_pool(name="xstart", bufs=1) as xstart_pool:
                # Startup phase x: per-k-block [128, 128] tiles so the first
                # matmul only waits for ~1.1 MB (w0 + 4 small x blocks), not
                # whole 2 MB x tiles behind the weight stream.
                xs_sb = {}
                for db in range(DB):
                    for mt in range(2):
                        for pi in range(2):
                            t = xstart_pool.tile(
                                [P, P],
                                mybir.dt.float16,
                                name=f"xs{db}_{mt}_{pi}",
                                tag=f"xs{db}_{mt}_{pi}",
                            )
                            nc.sync.dma_start(out=t[:], in_=xt_pair_d[db, mt, pi])
                            xs_sb[db, mt, pi] = t
                    # interleave weight blocks with startup-x so w[db] arrives
                    # roughly when the PE needs it
                    w = wpool.tile(
                        [P, O_SH], mybir.dt.float16, name=f"w{db}", tag=f"w{db}"
                    )
                    nc.sync.dma_start(out=w[:], in_=wt_d[db])
                    w_sb.append(w)
                bias_sb = wpool.tile([P, O_SH], mybir.dt.float32, name="bias_sb")
                nc.sync.dma_start(out=bias_sb[:], in_=bias_d[:])

                # Prefetch x for m-tiles 2..3 now: their dma_starts must sit
                # ahead of the pair-phase eviction DMAs in the in-order sync
                # stream, else they head-of-line block until the pair phase
                # fully drains.
                x_next = {mt: load_x(mt) for mt in (2, 3)}

                # Startup phase: m-tiles 0+1 jointly, k-block-major, so PE
                # consumption (~3.4us per k-block) paces the weight stream
                # (~3.1us per block) instead of stalling for the full preload.
                psums01 = [alloc_psums(0), alloc_psums(1)]
                for db in range(DB):
                    for mt in range(2):
                        for pi in range(2):
                            lhsT = xs_sb[db, mt, pi]
                            for oc in range(NCH):
                                nc.tensor.matmul(
                                    psums01[mt][oc][:],
                                    lhsT[:],
                                    w_sb[db][:, oc * NF : (oc + 1) * NF],
                                    start=db == 0 and pi == 0,
                                    stop=db == DB - 1 and pi == 1,
                                )
            # opool created only after xstart is released so their SBUF
            # address ranges can overlap in time.
            with tc.tile_pool(name="opool", bufs=2) as opool:
                for mt in range(2):
                    evict(opool, mt, psums01[mt])

                # Steady state: one m-tile at a time, double-buffered.
                for mt in range(2, MT):
                    x_pair = x_next.pop(mt) if mt in x_next else load_x(mt)
                    psums = alloc_psums(mt)
                    if mt < MT - 1:
                        for db in range(DB):
                            mm_block(x_pair, psums, db, db == 0, db == DB - 1)
                        evict(opool, mt, psums)
                    else:
                        # Last m-tile: oc-major so each output chunk's 64-matmul
                        # accumulation finishes (and evicts) as early as
                        # possible instead of all four at the very end.
                        for oc in range(NCH):
                            for db in range(DB):
                                for i, x_sb in enumerate(x_pair):
                                    nc.tensor.matmul(
                                        psums[oc][:],
                                        x_sb[:, db * P : (db + 1) * P],
                                        w_sb[db][:, oc * NF : (oc + 1) * NF],
                                        start=db == 0 and i == 0,
                                        stop=db == DB - 1 and i == 1,
                                    )
                            o_sb = opool.tile(
                                [P, NF], mybir.dt.float32, name="o_sb", tag=f"o{oc}"
                            )
                            nc.vector.tensor_add(
                                o_sb[:],
                                psums[oc][:],
                                bias_sb[:, oc * NF : (oc + 1) * NF],
                            )
                            nc.sync.dma_start(
                                out=out_d[
                                    mt * P : (mt + 1) * P, oc * NF : (oc + 1) * NF
                                ],
                                in_=o_sb[:],
                            )
    nc.compile()
    return nc


def _prep_inputs(x, weight, bias):
    xf = np.ascontiguousarray(x.reshape(M_TOT, D_IN), dtype=np.float32)
    x_hi = xf.astype(np.float16)
    x_lo = (xf - x_hi.astype(np.float32)).astype(np.float16)

    qw = np.sign(weight.astype(np.float32)).astype(np.float16)  # [o, d]

    # per o-group weight block + broadcast bias, shared by all cores in group
    wt_og, bias_og = [], []
    for og in range(OG):
        o0 = og * O_SH
        blk = np.ascontiguousarray(qw[o0 : o0 + O_SH, :].T)  # [d, o]
        wt_og.append(blk.reshape(DB, P, O_SH))
        bb = np.ascontiguousarray(
            np.broadcast_to(bias[o0 : o0 + O_SH].astype(np.float32), (P, O_SH))
        )
        bias_og.append(bb)

    in_maps = []
    for c in range(N_CORES):
        mg, og = c // OG, c % OG
        m0 = mg * M_SH
        xt = {}
        r4 = {}
        for name, arr in (("xt_hi", x_hi), ("xt_lo", x_lo)):
            r = arr[m0 : m0 + M_SH].reshape(MT, P, DB, P)  # [mt, m, db, d]
            r4[name] = np.ascontiguousarray(r.transpose(0, 3, 2, 1))  # [mt,d,db,m]
            xt[name] = r4[name].reshape(MT, P, DB * P)
        # startup-phase copy of m-tiles 0..1 in k-block-major order
        xt_pair = np.empty((DB, 2, 2, P, P), dtype=np.float16)
        for pi, name in enumerate(("xt_hi", "xt_lo")):
            xt_pair[:, :, pi] = r4[name][:2].transpose(2, 0, 1, 3)  # [db, mt, d, m]
        in_maps.append(
            {
                "xt_hi": xt["xt_hi"],
                "xt_lo": xt["xt_lo"],
                "xt_pair": xt_pair,
                "wt": wt_og[og],
                "biasb": bias_og[og],
            }
        )
    return in_maps


def run(inputs, trace=False):
    """Run the SPMD kernel; returns (full_output, BassKernelResults)."""
    if "nc" not in _CACHE:
        _CACHE["nc"] = _build()
    nc = _CACHE["nc"]
    in_maps = _prep_inputs(inputs["x"], inputs["weight"], inputs["bias"])
    res = run_bass_kernel_spmd(nc, in_maps, list(range(N_CORES)), trace=trace)
    out = np.empty((M_TOT, D_OUT), dtype=np.float32)
    for c in range(N_CORES):
        mg, og = c // OG, c % OG
        out[mg * M_SH : (mg + 1) * M_SH, og * O_SH : (og + 1) * O_SH] = res.results[
            c
        ]["out"]
    return out.reshape(B, S, D_OUT), res


def kernel(x, weight, bias):
    out, _ = run({"x": x, "weight": weight, "bias": bias})
    return out
